# revision 1
# baseline (speedup 1.0000x reference)
"""GAT (3-layer, 4-head) + global-max-pool + MLP on 8 Trainium2 NeuronCores.

Strategy (graph/data parallel per the sharding hint):
  - Destination nodes are sharded 6250/core; each core owns the segment
    softmax + aggregation for its nodes (1D edge cut by dst).
  - Per layer, a "table" of node payload rows [h|alpha_src|alpha_dst] (512B,
    fp16 h + f32 alphas) is replicated to every core's HBM; per-edge h[src]
    rows are fetched with dma_gather (int16 indices -> lo/hi table split at
    row 32768; <=1024 indices per instruction, the 64-desc/engine packet cap).
  - Edges live in an ELL layout: local dst nodes sorted by (total, hi)
    degree, tiles of 128 nodes x K slots (K = per-tile max degree, shared
    across cores for SPMD).  alpha_dst is per-partition -> cheap broadcast
    adds; padding slots are masked with -60000 before the per-node-max
    subtraction and exp (LeakyReLU is computed on DVE as max(x, 0.2x); the
    ACT Lrelu table ignores its alpha argument).
  - Aggregation: per-slot matmul(lhsT=identity_f16, rhs=v_k) accumulating
    into PSUM (segment-sum on the TensorEngine, node-major output, no
    weight reloads).
  - Halo exchange: collective AllGather of each core's 3.2MB table slab
    (~80us for the full 25.7MB on this fabric).
  - Pooling: h3 staged node-major in HBM, graph-slot dma_gather,
    PE-identity transpose to hc-major, per-graph reduce_max over static
    ranges, AllReduce(max), tiny on-device MLP.  Empty graphs hit the
    zero sentinel row, matching the reference's isfinite guard.

  The lo/hi gather windows OVERLAP in table rows [17408, 32768): edges
  whose source sid falls there are assigned to whichever half balances the
  dst node's lo/hi counts (padding 1.52x -> 1.28x).

  Measured on this fabric: relative L2 error 1.3e-4; est. device time
  ~4.05 ms/exec (gather-descriptor latency bound: SWDGE processes one
  512B descriptor per SDMA engine at a time, ~10 ns/descriptor; 4-deep
  gather double-buffering keeps the SDMA queues continuously fed).
"""

import warnings

warnings.filterwarnings("ignore")

import numpy as np

# ---------------- problem constants (from spec) ----------------
N = 50000
E = 800000
F = 128
H = 4
C = 32
HC = 128
LIN = 256
OUTD = 10
G = 64
NEG = 0.2

NCORES = 8
NLOC = N // NCORES            # 6250 local dst nodes per core
NTILE = 49                    # ceil(6250/128)
NPAD = NTILE * 128            # 6272 rows per core slab
TROWS = NCORES * NPAD         # 50176 table rows
LOCUT = 32768                 # lo gather window = table rows [0, 32768)
HIBASE = 17408                # hi gather window = rows [17408, 50176)
NIDX_MAX = 1024               # max indices per dma_gather instruction
MASK_NEG = -60000.0

_COMPILED = {}
DEBUG_OUTS = False


def _f16(x):
    """f32 -> fp16 bit pattern, as uint16."""
    return np.asarray(x, np.float32).astype(np.float16).view(np.uint16)


def _wrap16(flat):
    """Wrap an int16 index list into the [128, n/16] SWDGE layout:
    index j lives at partition j%16, column j//16; replicated across the
    eight 16-partition groups (one per Q7 core)."""
    flat = np.asarray(flat, np.int16)
    assert len(flat) % 16 == 0
    a = np.empty((128, len(flat) // 16), np.int16)
    blk = flat.reshape(-1, 16).T
    for g_ in range(8):
        a[g_ * 16:(g_ + 1) * 16, :] = blk
    return a


def _pack_table_rows(h, a_s, a_d, sid, nrows):
    """Build table rows [h fp16 x128 | a_s f32 x4 | a_d f32 x4 | pad] = 512B,
    written at row positions sid."""
    n = h.shape[0]
    tab = np.zeros((nrows, 128), np.float32)
    hb = _f16(h)
    words = hb[:, 0::2].astype(np.uint32) | (hb[:, 1::2].astype(np.uint32) << 16)
    tw = tab.view(np.uint32)
    tw[sid, 0:64] = words
    tab[sid, 64:68] = np.asarray(a_s, np.float32)
    tab[sid, 68:72] = np.asarray(a_d, np.float32)
    return tab


def _host_prep(inputs):
    x = np.asarray(inputs["x"], np.float32)
    ei = np.asarray(inputs["edge_index"]).astype(np.int64)
    batch = np.asarray(inputs["batch"]).astype(np.int64)

    src = np.concatenate([ei[0], np.arange(N, dtype=np.int64)])
    dst = np.concatenate([ei[1], np.arange(N, dtype=np.int64)])

    indeg = np.bincount(dst, minlength=N)

    def _sort_cores(keyfun):
        sortpos = np.empty(N, np.int64)
        perm = np.empty((NCORES, NLOC), np.int64)
        for c in range(NCORES):
            a = c * NLOC
            order = keyfun(a)
            perm[c] = order
            sortpos[a + order] = np.arange(NLOC)
        sid = (np.arange(N) // NLOC) * NPAD + sortpos
        return sortpos, perm, sid

    def _lohi(sid):
        """Assign each edge to the lo window [0,32768) or hi window
        [HIBASE,TROWS).  Sources with sid in the overlap [HIBASE,32768) are
        flexible; balance each dst node's lo/hi counts to minimize per-tile
        max_lo + max_hi padding."""
        s_sid = sid[src]
        f_lo = s_sid < HIBASE
        f_hi = s_sid >= LOCUT
        flex = ~f_lo & ~f_hi
        nfl = np.bincount(dst[f_lo], minlength=N)
        nfx = np.bincount(dst[flex], minlength=N)
        tgt = np.clip(np.round(indeg * 0.5).astype(np.int64), nfl, nfl + nfx)
        # rank flexible edges within each dst
        fe = np.where(flex)[0]
        fo = fe[np.argsort(dst[fe], kind="stable")]
        dsf = dst[fo]
        firsts = np.r_[True, dsf[1:] != dsf[:-1]]
        gs = np.maximum.accumulate(np.where(firsts, np.arange(len(fo)), 0))
        frank = np.arange(len(fo)) - gs
        is_lo = f_lo.copy()
        is_lo[fo] = frank < (tgt - nfl)[dsf]
        key = dst * 2 + (~is_lo).astype(np.int64)
        cnt = np.bincount(key, minlength=2 * N)
        return s_sid, is_lo, key, cnt[0::2], cnt[1::2]

    # phase 1: provisional sort by total degree -> provisional lo/hi counts
    sortpos, perm, sid = _sort_cores(
        lambda a: np.argsort(indeg[a:a + NLOC], kind="stable"))
    _, _, _, p_lo, p_hi = _lohi(sid)
    # phase 2: final sort by (total degree, hi-degree) for tight ELL tiles
    p_tot = p_lo + p_hi
    sortpos, perm, sid = _sort_cores(
        lambda a: np.lexsort((p_hi[a:a + NLOC], p_tot[a:a + NLOC])))
    s_sid, is_lo, key, n_lo, n_hi = _lohi(sid)
    d_core = dst // NLOC
    d_sp = sortpos[dst]                                     # sorted pos of dst

    # per-tile K (max over nodes in tile AND over cores, for SPMD)
    n_lo_s = np.zeros((NCORES, NPAD), np.int64)
    n_hi_s = np.zeros((NCORES, NPAD), np.int64)
    for c in range(NCORES):
        a = c * NLOC
        n_lo_s[c, sortpos[a:a + NLOC]] = n_lo[a:a + NLOC]
        n_hi_s[c, sortpos[a:a + NLOC]] = n_hi[a:a + NLOC]
    K1 = n_lo_s.reshape(NCORES, NTILE, 128).max(axis=(0, 2))   # [NTILE]
    K2 = n_hi_s.reshape(NCORES, NTILE, 128).max(axis=(0, 2))
    KT = K1 + K2

    # rank of each edge within its (dst, half) group
    eorder = np.lexsort((~is_lo, d_sp, d_core))
    so_key = key[eorder]
    firsts = np.r_[True, so_key[1:] != so_key[:-1]]
    grp_start = np.maximum.accumulate(np.where(firsts, np.arange(len(eorder)), 0))
    rank_sorted = np.arange(len(eorder)) - grp_start
    rank = np.empty(len(eorder), np.int64)
    rank[eorder] = rank_sorted

    # flat slot column for each edge: tile column base + (rank or K1+rank)
    coff = np.zeros(NTILE + 1, np.int64)
    coff[1:] = np.cumsum(KT)
    tot_slots = int(coff[-1])
    tile_of = d_sp // 128
    p_of = d_sp % 128
    k_of = rank + np.where(is_lo, 0, K1[tile_of])

    # per-core idx arrays (value in table space) + masks
    # slot (tile t, k, p) -> flat j index per tile-half chunking below
    idx_val = np.zeros((NCORES, tot_slots, 128), np.int16)   # [core, col, p]
    mask = np.full((NCORES, 128, tot_slots), MASK_NEG, np.float32)
    cols = coff[tile_of] + k_of
    val = np.where(is_lo, s_sid, s_sid - HIBASE).astype(np.int16)
    idx_val[d_core, cols, p_of] = val
    mask[d_core, p_of, cols] = 0.0

    # gather instruction schedule (static, shared by all cores):
    # per tile: lo chunks of <=8 slots, then hi chunks of <=8 slots
    sched = []          # (tile, slot_off_in_tile, nslots, idx_flat_off)
    off = 0
    for t in range(NTILE):
        for half, kk in ((0, int(K1[t])), (1, int(K2[t]))):
            s0 = 0 if half == 0 else int(K1[t])
            k = 0
            while k < kk:
                nk = min(8, kk - k)
                sched.append((t, half, s0 + k, nk, off))
                off += nk * 128
                k += nk
    tot_idx = off

    # flat idx buffers in j = k*128 + p order per chunk
    idx_wrapped = np.empty((NCORES, 128, tot_idx // 16), np.int16)
    for c in range(NCORES):
        flat = np.empty(tot_idx, np.int16)
        for (t, half, s0, nk, o) in sched:
            colbase = coff[t] + s0
            flat[o:o + nk * 128] = idx_val[c, colbase:colbase + nk].reshape(-1)
        idx_wrapped[c] = _wrap16(flat)

    # pooling: graph slot layout
    node_graph = batch                                       # [N]
    cnt_gc = np.zeros((G, NCORES), np.int64)
    for c in range(NCORES):
        cnt_gc[:, c] = np.bincount(batch[c * NLOC:(c + 1) * NLOC], minlength=G)
    PG = int(cnt_gc.max())
    PG = max(PG, 1)
    GB = 4                                 # graphs per pooling block
    NBLK = G // GB
    nb = ((GB * PG + 127) // 128) * 128    # slots per block (gather chunks)
    npool = NBLK * nb
    # slot value = sorted-local node row in the h3 staging, NPAD = zero row
    pool_idx = np.full((NCORES, npool), NPAD, np.int16)
    for c in range(NCORES):
        a = c * NLOC
        gl = batch[a:a + NLOC]
        order2 = np.argsort(gl, kind="stable")
        ranks = np.arange(NLOC) - np.maximum.accumulate(
            np.where(np.r_[True, gl[order2][1:] != gl[order2][:-1]],
                     np.arange(NLOC), 0))
        g_ = gl[order2]
        slots = (g_ // GB) * nb + (g_ % GB) * PG + ranks
        pool_idx[c, slots] = sortpos[a + order2]
    pool_wrapped = np.stack([_wrap16(pool_idx[c]) for c in range(NCORES)])

    # ---- weights / constants ----
    def aflat(a):
        m = np.zeros((128, H), np.float32)
        for h_ in range(H):
            m[h_ * C:(h_ + 1) * C, h_] = np.asarray(a, np.float32)[h_]
        return m

    W0 = np.asarray(inputs["W0"], np.float32)
    h0 = x @ W0
    h0r = h0.reshape(N, H, C)
    as0 = (h0r * np.asarray(inputs["as0"], np.float32)).sum(-1)
    ad0 = (h0r * np.asarray(inputs["ad0"], np.float32)).sum(-1)
    table0 = _pack_table_rows(h0, as0, ad0, sid, TROWS)

    # layer-0 alpha_dst per core in sorted order [128, NTILE*H]
    ad0_loc = np.zeros((NCORES, 128, NTILE * H), np.float32)
    for c in range(NCORES):
        a = c * NLOC
        tmp = np.zeros((NPAD, H), np.float32)
        tmp[sortpos[a:a + NLOC]] = ad0[a:a + NLOC]
        ad0_loc[c] = tmp.reshape(NTILE, 128, H).transpose(1, 0, 2).reshape(128, NTILE * H)

    consts = {
        "idx": idx_wrapped,                                  # per-core
        "mask": mask,                                        # per-core
        "pool_idx": pool_wrapped,                            # per-core
        "table0": table0,                                    # replicated
        "ad0_loc": ad0_loc,                                  # per-core
        "W1": np.asarray(inputs["W1"], np.float32),
        "W2": np.asarray(inputs["W2"], np.float32),
        # alphas are dot products against h = x@W, so fold W in:
        "afs1": np.asarray(inputs["W1"], np.float32) @ aflat(inputs["as1"]),
        "afd1": np.asarray(inputs["W1"], np.float32) @ aflat(inputs["ad1"]),
        "afs2": np.asarray(inputs["W2"], np.float32) @ aflat(inputs["as2"]),
        "afd2": np.asarray(inputs["W2"], np.float32) @ aflat(inputs["ad2"]),
        "b0c": np.tile(np.asarray(inputs["b0"], np.float32), (128, 1)),
        "b1c": np.tile(np.asarray(inputs["b1"], np.float32), (128, 1)),
        "b2c": np.tile(np.asarray(inputs["b2"], np.float32), (128, 1)),
        "identf": np.eye(128, dtype=np.float32),
        "identb": np.eye(128, dtype=np.float32),   # cast to bf16 at feed time
        "Wlin": np.asarray(inputs["Wlin"], np.float32),
        "blinc": np.tile(np.asarray(inputs["blin"], np.float32), (64, 1)),
        "Wout": np.asarray(inputs["Wout"], np.float32),
        "boutc": np.tile(np.asarray(inputs["bout"], np.float32), (64, 1)),
    }
    meta = dict(K1=[int(v) for v in K1], K2=[int(v) for v in K2],
                KT=[int(v) for v in KT], coff=[int(v) for v in coff],
                sched=sched, tot_idx=tot_idx, tot_slots=tot_slots,
                PG=PG, GB=GB, nb=nb, npool=npool)
    return consts, meta


# ---------------------------------------------------------------------------
def _build_module(meta):
    import concourse.bacc as bacc
    import concourse.bass as bass
    import concourse.mybir as mybir
    import concourse.tile as tile

    dtf = mybir.dt.float32
    dtb = mybir.dt.float16
    AF = mybir.ActivationFunctionType
    K1, K2, KT = meta["K1"], meta["K2"], meta["KT"]
    coff, sched = meta["coff"], meta["sched"]
    KMAX = max(KT)
    PG, GB, nb, npool = meta["PG"], meta["GB"], meta["nb"], meta["npool"]

    nc = bacc.Bacc("TRN2", target_bir_lowering=False, debug=False,
                   num_devices=NCORES)

    # ---- I/O ----
    t_idx = nc.dram_tensor("idx", [128, meta["tot_idx"] // 16], mybir.dt.int16,
                           kind="ExternalInput")
    t_mask = nc.dram_tensor("mask", [128, meta["tot_slots"]], dtb,
                            kind="ExternalInput")
    t_pool = nc.dram_tensor("pool_idx", [128, npool // 16], mybir.dt.int16,
                            kind="ExternalInput")
    t_tab0 = nc.dram_tensor("table0", [TROWS, 128], dtf, kind="ExternalInput")
    t_ad0 = nc.dram_tensor("ad0_loc", [128, NTILE * H], dtf, kind="ExternalInput")
    t_identb = nc.dram_tensor("identb", [128, 128], dtb, kind="ExternalInput")
    t_identf = nc.dram_tensor("identf", [128, 128], dtf, kind="ExternalInput")
    ins = {}
    for nm, shp in (("W1", [128, 128]), ("W2", [128, 128]),
                    ("afs1", [128, H]), ("afd1", [128, H]),
                    ("afs2", [128, H]), ("afd2", [128, H]),
                    ("b0c", [128, 128]), ("b1c", [128, 128]), ("b2c", [128, 128]),
                    ("Wlin", [128, LIN]), ("blinc", [64, LIN]),
                    ("Wout", [LIN, OUTD]), ("boutc", [64, OUTD])):
        ins[nm] = nc.dram_tensor(nm, shp, dtf, kind="ExternalInput")
    t_out = nc.dram_tensor("out", [64, OUTD], dtf, kind="ExternalOutput")
    dbg = {}
    if DEBUG_OUTS:
        for l_ in range(3):
            dbg[f"x1_l{l_}"] = nc.dram_tensor(f"dbg_x1_l{l_}", [NPAD, 128], dtf,
                                              kind="ExternalOutput")
        dbg["tab1"] = nc.dram_tensor("dbg_tab1", [NPAD, 128], dtf,
                                     kind="ExternalOutput")
        dbg["ad1"] = nc.dram_tensor("dbg_ad1", [128, NTILE * H], dtf,
                                    kind="ExternalOutput")
        dbg["pooledT"] = nc.dram_tensor("dbg_pooledT", [128, 64], dtf,
                                        kind="ExternalOutput")

    with tile.TileContext(nc) as tc:
        with (
            tc.tile_pool(name="const", bufs=1) as constp,
            tc.tile_pool(name="gslab", bufs=4) as gpool,
            tc.tile_pool(name="vslab", bufs=3) as vpool,
            tc.tile_pool(name="small", bufs=3) as spool,
            tc.tile_pool(name="node", bufs=3) as npool_sb,
            tc.tile_pool(name="keep", bufs=1) as keep,
            tc.tile_pool(name="gpp", bufs=1) as gpp,
            tc.tile_pool(name="ps", bufs=2, space="PSUM") as psp,
            tc.tile_pool(name="ps2", bufs=1, space="PSUM") as psp2,
            tc.tile_pool(name="dram", bufs=1, space="DRAM") as dram,
        ):
            # ---- persistent SBUF ----
            idx_sb = constp.tile([128, meta["tot_idx"] // 16], mybir.dt.int16)
            nc.sync.dma_start(idx_sb[:], t_idx[:])
            mask_sb = constp.tile([128, meta["tot_slots"]], dtb)
            nc.sync.dma_start(mask_sb[:], t_mask[:])
            pool_sb = constp.tile([128, npool // 16], mybir.dt.int16)
            nc.sync.dma_start(pool_sb[:], t_pool[:])
            ident_b = constp.tile([128, 128], dtb)
            ident_f = constp.tile([128, 128], dtf)
            nc.sync.dma_start(ident_b[:], t_identb[:])
            nc.sync.dma_start(ident_f[:], t_identf[:])
            csb = {}
            for nm in ("W1", "W2", "afs1", "afd1", "afs2", "afd2",
                       "b0c", "b1c", "b2c", "Wlin", "blinc", "boutc"):
                csb[nm] = constp.tile(list(ins[nm].shape), dtf, tag=nm, name=nm)
                nc.sync.dma_start(csb[nm][:], ins[nm][:])
            wout_sb = []
            for j in range(2):
                w = constp.tile([128, OUTD], dtf, tag=f"wout{j}", name=f"wout{j}")
                nc.sync.dma_start(w[:], ins["Wout"][j * 128:(j + 1) * 128, :])
                wout_sb.append(w)

            # alpha_dst for current layer, [128, NTILE*H]
            ad_cur = keep.tile([128, NTILE * H], dtf, tag="ad_cur")
            nc.sync.dma_start(ad_cur[:], t_ad0[:])
            ad_next = keep.tile([128, NTILE * H], dtf, tag="ad_next")

            # ---- DRAM tables ----
            tables = [t_tab0]
            slabs = []
            for l_ in (1, 2):
                tables.append(dram.tile([TROWS, 128], dtf, addr_space="Shared",
                                        tag=f"tab{l_}", name=f"tab{l_}"))
                slabs.append(dram.tile([NPAD, 128], dtf, tag=f"slab{l_}",
                                       name=f"slab{l_}"))
            h3_stage = dram.tile([NPAD + 128, 128], dtf, tag="h3st",
                                 name="h3st")

            layer_w = {0: ("W1", "afs1", "afd1", "b0c"),
                       1: ("W2", "afs2", "afd2", "b1c"),
                       2: (None, None, None, "b2c")}

            for l_ in range(3):
                tabl = tables[l_]
                wname, asname, adname, bname = layer_w[l_]
                sched_by_tile = {}
                for (t, half, s0, nk, o) in sched:
                    sched_by_tile.setdefault(t, []).append((half, s0, nk, o))

                for t in range(NTILE):
                    kt, k1 = KT[t], K1[t]
                    # ---- gather payload rows into ELL slab ----
                    Gt = gpool.tile([128, KMAX, 128], dtf, tag="G")
                    for (half, s0, nk, o) in sched_by_tile[t]:
                        srcap = tabl[:LOCUT, :] if half == 0 else tabl[HIBASE:, :]
                        nc.gpsimd.dma_gather(
                            out_ap=Gt[:, s0:s0 + nk, :],
                            in_ap=srcap,
                            idxs_ap=idx_sb[:, o // 16:(o + nk * 128) // 16],
                            num_idxs=nk * 128,
                            num_idxs_reg=nk * 128,
                            elem_size=128,
                        )
                    # views
                    g_bf = Gt[:].bitcast(dtb)                  # [128, KMAX, 256]
                    h_view = g_bf[:, 0:kt, 0:128].rearrange(
                        "p k (h c) -> p k h c", h=H)
                    as_view = Gt[:, 0:kt, 64:64 + H]           # f32 [128,kt,4]

                    # ---- attention ----
                    e_t = spool.tile([128, KMAX, H], dtf, tag="e")
                    ad_b = ad_cur[:, t * H:(t + 1) * H].rearrange(
                        "p (u h) -> p u h", u=1).broadcast_to([128, kt, H])
                    nc.vector.tensor_tensor(
                        out=e_t[:, 0:kt, :], in0=as_view, in1=ad_b,
                        op=mybir.AluOpType.add)
                    # LeakyReLU = max(x, 0.2x) on DVE (ACT Lrelu table bakes
                    # its own slope and ignores alpha)
                    lr_t = spool.tile([128, KMAX, H], dtf, tag="lr")
                    nc.vector.tensor_scalar_mul(lr_t[:, 0:kt, :],
                                                e_t[:, 0:kt, :], NEG)
                    nc.vector.tensor_tensor(out=e_t[:, 0:kt, :],
                                            in0=e_t[:, 0:kt, :],
                                            in1=lr_t[:, 0:kt, :],
                                            op=mybir.AluOpType.max)
                    m_b = mask_sb[:, coff[t]:coff[t] + kt].rearrange(
                        "p (k u) -> p k u", u=1).broadcast_to([128, kt, H])
                    nc.vector.tensor_tensor(
                        out=e_t[:, 0:kt, :], in0=e_t[:, 0:kt, :], in1=m_b,
                        op=mybir.AluOpType.add)
                    mx = spool.tile([128, H], dtf, tag="mx")
                    nc.vector.tensor_reduce(
                        out=mx[:],
                        in_=e_t[:, 0:kt, :].rearrange("p k h -> p h k"),
                        axis=mybir.AxisListType.X, op=mybir.AluOpType.max)
                    mx_b = mx[:].rearrange("p (u h) -> p u h", u=1).broadcast_to(
                        [128, kt, H])
                    nc.vector.tensor_tensor(out=e_t[:, 0:kt, :],
                                            in0=e_t[:, 0:kt, :], in1=mx_b,
                                            op=mybir.AluOpType.subtract)
                    ex_t = spool.tile([128, KMAX, H], dtb, tag="ex")
                    nc.scalar.activation(ex_t[:, 0:kt, :], e_t[:, 0:kt, :],
                                         AF.Exp)

                    # ---- denominators ----
                    den = spool.tile([128, H], dtf, tag="den")
                    nc.vector.tensor_reduce(
                        out=den[:],
                        in_=ex_t[:, 0:kt, :].rearrange("p k h -> p h k"),
                        axis=mybir.AxisListType.X, op=mybir.AluOpType.add)
                    nc.vector.tensor_scalar_max(den[:], den[:], 1e-30)
                    rec = spool.tile([128, H], dtf, tag="rec")
                    nc.vector.reciprocal(rec[:], den[:])

                    # ---- weighted values ----
                    v_t = vpool.tile([128, KMAX, H, C], dtb, tag="v")
                    ex_b = ex_t[:, 0:kt, :].rearrange(
                        "p k (h u) -> p k h u", u=1).broadcast_to([128, kt, H, C])
                    nc.vector.tensor_tensor(out=v_t[:, 0:kt, :, :],
                                            in0=h_view, in1=ex_b,
                                            op=mybir.AluOpType.mult)

                    # ---- aggregation: Num[d, hc] = sum_k v_k ----
                    num_ps = psp.tile([128, 128], dtf, tag="num")
                    vflat = v_t[:].rearrange("p k h c -> p k (h c)")
                    for k in range(kt):
                        nc.tensor.matmul(num_ps[:], lhsT=ident_b[:],
                                         rhs=vflat[:, k, :],
                                         start=(k == 0), stop=(k == kt - 1))

                    # ---- normalize + bias + relu -> x1 [d, hc] f32 ----
                    x1 = npool_sb.tile([128, 128], dtf, tag="x1")
                    rec_b = rec[:].rearrange("p (h u) -> p h u", u=1).broadcast_to(
                        [128, H, C])
                    nc.vector.tensor_tensor(
                        out=x1[:].rearrange("p (h c) -> p h c", h=H),
                        in0=num_ps[:].rearrange("p (h c) -> p h c", h=H),
                        in1=rec_b, op=mybir.AluOpType.mult)
                    nc.vector.tensor_tensor(out=x1[:], in0=x1[:],
                                            in1=csb[bname][:],
                                            op=mybir.AluOpType.add)
                    nc.vector.tensor_scalar_max(x1[:], x1[:], 0.0)
                    if DEBUG_OUTS:
                        nc.sync.dma_start(
                            dbg[f"x1_l{l_}"][t * 128:(t + 1) * 128, :], x1[:])

                    if l_ < 2:
                        # ---- transpose x1 -> x1T [hc, d] ----
                        x1T_ps = psp2.tile([128, 128], dtf, tag="x1T")
                        nc.tensor.matmul(x1T_ps[:], lhsT=x1[:],
                                         rhs=ident_f[:], start=True, stop=True)
                        x1T = npool_sb.tile([128, 128], dtf, tag="x1T_sb")
                        nc.scalar.copy(x1T[:], x1T_ps[:])
                        # ---- node pass: table row for next layer ----
                        row_ps = psp.tile([128, 136], dtf, tag="row")
                        nc.tensor.matmul(row_ps[:, 0:128], lhsT=x1T[:],
                                         rhs=csb[wname][:], start=True, stop=True)
                        nc.tensor.matmul(row_ps[:, 128:132], lhsT=x1T[:],
                                         rhs=csb[asname][:], start=True, stop=True)
                        nc.tensor.matmul(row_ps[:, 132:136], lhsT=x1T[:],
                                         rhs=csb[adname][:], start=True, stop=True)
                        slab_sb = npool_sb.tile([128, 128], dtf, tag="slabrow")
                        slab_bf = slab_sb[:].bitcast(dtb)      # [128, 256]
                        nc.scalar.copy(slab_bf[:, 0:128], row_ps[:, 0:128])
                        nc.vector.tensor_copy(slab_sb[:, 64:72],
                                              row_ps[:, 128:136])
                        nc.vector.tensor_copy(
                            ad_next[:, t * H:(t + 1) * H], row_ps[:, 132:136])
                        nc.sync.dma_start(
                            slabs[l_][t * 128:(t + 1) * 128, :], slab_sb[:])
                    else:
                        nc.sync.dma_start(
                            h3_stage[t * 128:(t + 1) * 128, :], x1[:])

                if l_ < 2:
                    nc.gpsimd.collective_compute(
                        "AllGather", mybir.AluOpType.bypass,
                        replica_groups=[list(range(NCORES))],
                        ins=[slabs[l_].opt()], outs=[tables[l_ + 1].opt()],
                    )
                    if DEBUG_OUTS and l_ == 0:
                        nc.gpsimd.dma_start(dbg["tab1"][:],
                                            tables[1][0:NPAD, :])
                        dsb = keep.tile([128, NTILE * H], dtf, tag="dbgad")
                        nc.vector.tensor_copy(dsb[:], ad_next[:])
                        nc.sync.dma_start(dbg["ad1"][:], dsb[:])
                    ad_cur, ad_next = ad_next, ad_cur

            # ---------------- pooling + MLP ----------------
            zrow = keep.tile([128, 128], dtf, tag="zrow")
            nc.vector.memset(zrow[:], 0.0)
            nc.sync.dma_start(h3_stage[NPAD:NPAD + 128, :], zrow[:])
            pooledT = keep.tile([128, 64], dtf, tag="pooledT")
            NCHK = nb // 128
            for b in range(G // GB):
                gt = gpp.tile([128, NCHK, 128], dtf, tag="gpool")
                o0 = b * nb
                k = 0
                while k < NCHK:
                    nk = min(8, NCHK - k)
                    nc.gpsimd.dma_gather(
                        out_ap=gt[:, k:k + nk, :],
                        in_ap=h3_stage[:],
                        idxs_ap=pool_sb[:, (o0 + k * 128) // 16:
                                        (o0 + (k + nk) * 128) // 16],
                        num_idxs=nk * 128, num_idxs_reg=nk * 128,
                        elem_size=128)
                    k += nk
                gp_sb = gpp.tile([128, nb], dtf, tag="gpsb")
                for j in range(NCHK):
                    tp = psp2.tile([128, 128], dtf, tag="x1T")
                    nc.tensor.matmul(tp[:], lhsT=gt[:, j, :], rhs=ident_f[:],
                                     start=True, stop=True)
                    nc.scalar.copy(gp_sb[:, j * 128:(j + 1) * 128], tp[:])
                for gi in range(GB):
                    nc.vector.tensor_reduce(
                        out=pooledT[:, b * GB + gi:b * GB + gi + 1],
                        in_=gp_sb[:, gi * PG:gi * PG + PG],
                        axis=mybir.AxisListType.X, op=mybir.AluOpType.max)

            if DEBUG_OUTS:
                nc.sync.dma_start(dbg["pooledT"][:], pooledT[:])
            # cross-core max
            ar_in = dram.tile([128, 64], dtf, tag="arin")
            ar_out = dram.tile([128, 64], dtf, addr_space="Shared", tag="arout")
            nc.sync.dma_start(ar_in[:], pooledT[:])
            nc.gpsimd.collective_compute(
                "AllReduce", mybir.AluOpType.max,
                replica_groups=[list(range(NCORES))],
                ins=[ar_in.opt()], outs=[ar_out.opt()],
            )
            pooled_sb = keep.tile([128, 64], dtf, tag="pooled2")
            nc.sync.dma_start(pooled_sb[:], ar_out[:])

            # z = pooled @ Wlin + blin  -> [64, 256]
            z_ps = psp2.tile([64, LIN], dtf, tag="z")
            nc.tensor.matmul(z_ps[:], lhsT=pooled_sb[:],
                             rhs=csb["Wlin"][:], start=True, stop=True)
            z_sb = keep.tile([64, LIN], dtf, tag="zsb")
            nc.vector.tensor_tensor(out=z_sb[:], in0=z_ps[:],
                                    in1=csb["blinc"][:],
                                    op=mybir.AluOpType.add)
            # zT (two 128-chunks)
            out_ps = psp2.tile([64, OUTD], dtf, tag="o")
            for j in range(2):
                zT_ps = psp2.tile([128, 64], dtf, tag="zT")
                nc.tensor.matmul(zT_ps[:], lhsT=z_sb[:, j * 128:(j + 1) * 128],
                                 rhs=ident_f[0:64, 0:64], start=True, stop=True)
                zT_sb = keep.tile([128, 64], dtf, tag=f"zTsb{j}")
                nc.scalar.copy(zT_sb[:], zT_ps[:])
                nc.tensor.matmul(out_ps[:], lhsT=zT_sb[:],
                                 rhs=wout_sb[j][:], start=(j == 0), stop=(j == 1))
            out_sb = keep.tile([64, OUTD], dtf, tag="osb")
            nc.vector.tensor_tensor(out=out_sb[:], in0=out_ps[:],
                                    in1=csb["boutc"][:], op=mybir.AluOpType.add)
            nc.sync.dma_start(t_out[:], out_sb[:])

    nc.compile()
    return nc


def kernel(**inputs):
    consts, meta = _host_prep(inputs)

    key = (meta["tot_idx"], meta["tot_slots"], meta["PG"], tuple(meta["KT"]))
    if key not in _COMPILED:
        _COMPILED[key] = _build_module(meta)
    nc = _COMPILED[key]

    in_maps = []
    for c in range(NCORES):
        m = {}
        for nm, v in consts.items():
            if nm in ("idx", "mask", "pool_idx", "ad0_loc"):
                m[nm] = np.ascontiguousarray(v[c])
            else:
                m[nm] = v
        m["mask"] = m["mask"].astype(np.float16)
        m["identb"] = m["identb"].astype(np.float16)
        in_maps.append(m)

    from concourse import bass2jax
    res = bass2jax.run_bass_via_pjrt(nc, in_maps, n_cores=NCORES)
    return np.asarray(res[0]["out"], np.float32)



# revision 47
# speedup vs baseline: 2.8566x; 2.8566x over previous
"""GAT (3-layer, 4-head) + global-max-pool + MLP on 8 Trainium2 NeuronCores.

Strategy (graph/data parallel per the sharding hint):
  - Destination nodes are sharded 6250/core; each core owns the segment
    softmax + aggregation for its nodes (1D edge cut by dst).
  - Per layer, a "table" of node payload rows [h|alpha_src|alpha_dst] (512B,
    fp16 h + f32 alphas) is replicated to every core's HBM; per-edge h[src]
    rows are fetched with dma_gather (int16 indices -> lo/hi table split at
    row 32768; <=1024 indices per instruction, the 64-desc/engine packet cap).
  - Edges live in an ELL layout: local dst nodes sorted by (total, hi)
    degree, tiles of 128 nodes x K slots (K = per-tile max degree, shared
    across cores for SPMD).  alpha_dst is per-partition -> cheap broadcast
    adds; padding slots are masked with -60000 before the per-node-max
    subtraction and exp (LeakyReLU is computed on DVE as max(x, 0.2x); the
    ACT Lrelu table ignores its alpha argument).
  - Aggregation: per-slot matmul(lhsT=identity_f16, rhs=v_k) accumulating
    into PSUM (segment-sum on the TensorEngine, node-major output, no
    weight reloads).
  - Halo exchange: collective AllGather of each core's 3.2MB table slab
    (~80us for the full 25.7MB on this fabric).
  - Pooling: h3 staged node-major in HBM, graph-slot dma_gather,
    PE-identity transpose to hc-major, per-graph reduce_max over static
    ranges, AllReduce(max), tiny on-device MLP.  Empty graphs hit the
    zero sentinel row, matching the reference's isfinite guard.

  The lo/hi gather windows OVERLAP in table rows [17408, 32768): edges
  whose source sid falls there are assigned to whichever half balances the
  dst node's lo/hi counts (padding 1.52x -> 1.28x).

  Measured on this fabric: relative L2 error 1.3e-4; est. device time
  ~4.05 ms/exec (gather-descriptor latency bound: SWDGE processes one
  512B descriptor per SDMA engine at a time, ~10 ns/descriptor; 4-deep
  gather double-buffering keeps the SDMA queues continuously fed).
"""

import warnings

warnings.filterwarnings("ignore")

import numpy as np

# ---------------- problem constants (from spec) ----------------
N = 50000
E = 800000
F = 128
H = 4
C = 32
HC = 128
LIN = 256
OUTD = 10
G = 64
NEG = 0.2

NCORES = 8
NLOC = N // NCORES            # 6250 local dst nodes per core
NTILE = 49                    # ceil(6250/128)
NPAD = NTILE * 128            # 6272 rows per core slab
TROWS = NCORES * NPAD         # 50176 table rows
LOCUT = 32768                 # lo gather window = table rows [0, 32768)
HIBASE = 17408                # hi gather window = rows [17408, 50176)
MASK_NEG = -60000.0
# Sentinel rows for ELL padding slots: slab pad rows (sorted positions
# >= NLOC) of core 0 (lo window) and core 7 (hi window).  Their alpha_src
# is forced to MASK_NEG so padded slots contribute exp(-big) = 0 without
# any mask tensor.
SENT_LO = NLOC                          # sid 6250, < LOCUT
SENT_HI = 7 * NPAD + NLOC - HIBASE      # sid 50154 -> hi idx 32746
PADP0 = NLOC - (NTILE - 1) * 128        # first pad partition in last tile

_COMPILED = {}
DEBUG_OUTS = False


def _f16(x):
    """f32 -> fp16 bit pattern, as uint16."""
    return np.asarray(x, np.float32).astype(np.float16).view(np.uint16)


def _wrap16(flat):
    """Wrap an int16 index list into the [128, n/16] SWDGE layout:
    index j lives at partition j%16, column j//16; replicated across the
    eight 16-partition groups (one per Q7 core)."""
    flat = np.asarray(flat, np.int16)
    assert len(flat) % 16 == 0
    a = np.empty((128, len(flat) // 16), np.int16)
    blk = flat.reshape(-1, 16).T
    for g_ in range(8):
        a[g_ * 16:(g_ + 1) * 16, :] = blk
    return a


def _pack_table_rows(h, a_s, sid, nrows):
    """Build table rows [h fp16 x128 | a_s fp16 x4 | pad] (512B container,
    words 0-65 meaningful), written at row positions sid.  Pad rows keep
    a_s = -60000 (the padding sentinel)."""
    n = h.shape[0]
    tab = np.zeros((nrows, 128), np.float32)
    sent = np.full(4, MASK_NEG, np.float16).view(np.uint32)
    tw = tab.view(np.uint32)
    tw[:, 64:66] = sent
    hb = _f16(h)
    words = hb[:, 0::2].astype(np.uint32) | (hb[:, 1::2].astype(np.uint32) << 16)
    tw[sid, 0:64] = words
    ab = _f16(a_s)
    tw[sid, 64:66] = (ab[:, 0::2].astype(np.uint32)
                      | (ab[:, 1::2].astype(np.uint32) << 16))
    return tab


def _host_prep(inputs):
    x = np.asarray(inputs["x"], np.float32)
    ei = np.asarray(inputs["edge_index"]).astype(np.int64)
    batch = np.asarray(inputs["batch"]).astype(np.int64)

    src = np.concatenate([ei[0], np.arange(N, dtype=np.int64)])
    dst = np.concatenate([ei[1], np.arange(N, dtype=np.int64)])

    indeg = np.bincount(dst, minlength=N)

    # Fixed pseudo-random balanced node->core assignment: decorrelates core
    # from graph id (batch is sorted, so id-blocks would put whole graphs on
    # one core and blow up the pooling layout) while keeping per-core degree
    # distributions iid for tight per-core ELL sorting.
    rng = np.random.default_rng(987654321)
    base_perm = rng.permutation(N)
    ncore = np.empty(N, np.int64)
    ncore[base_perm] = np.arange(N) % NCORES

    def _assign(keys):
        """Per-core stable lexsort by keys (last array = primary key)."""
        sortpos = np.empty(N, np.int64)
        for c in range(NCORES):
            nodes_c = np.flatnonzero(ncore == c)
            order = nodes_c[np.lexsort(tuple(k[nodes_c] for k in keys))]
            sortpos[order] = np.arange(NLOC)
        sid = ncore * NPAD + sortpos
        return sortpos, sid

    def _lohi(sid):
        """Assign each edge to the lo window [0,32768) or hi window
        [HIBASE,TROWS).  Sources with sid in the overlap [HIBASE,32768) are
        flexible; balance each dst node's lo/hi counts to minimize per-tile
        max_lo + max_hi padding."""
        s_sid = sid[src]
        f_lo = s_sid < HIBASE
        f_hi = s_sid >= LOCUT
        flex = ~f_lo & ~f_hi
        nfl = np.bincount(dst[f_lo], minlength=N)
        nfx = np.bincount(dst[flex], minlength=N)
        tgt = np.clip(np.round(indeg * 0.5).astype(np.int64), nfl, nfl + nfx)
        # rank flexible edges within each dst
        fe = np.where(flex)[0]
        fo = fe[np.argsort(dst[fe], kind="stable")]
        dsf = dst[fo]
        firsts = np.r_[True, dsf[1:] != dsf[:-1]]
        gs = np.maximum.accumulate(np.where(firsts, np.arange(len(fo)), 0))
        frank = np.arange(len(fo)) - gs
        is_lo = f_lo.copy()
        is_lo[fo] = frank < (tgt - nfl)[dsf]
        key = dst * 2 + (~is_lo).astype(np.int64)
        cnt = np.bincount(key, minlength=2 * N)
        return s_sid, is_lo, key, cnt[0::2], cnt[1::2]

    # phase 1: provisional per-core sort by total degree -> lo/hi counts
    sortpos, sid = _assign((indeg,))
    _, _, _, p_lo, p_hi = _lohi(sid)
    # phase 2: final per-core sort by (total degree, hi-degree)
    p_tot = p_lo + p_hi
    sortpos, sid = _assign((p_hi, p_tot))
    s_sid, is_lo, key, n_lo, n_hi = _lohi(sid)
    d_core = ncore[dst]
    d_sp = sortpos[dst]                                     # sorted pos of dst

    # per-tile K (max over nodes in tile AND over cores, for SPMD)
    n_lo_s = np.zeros((NCORES, NPAD), np.int64)
    n_hi_s = np.zeros((NCORES, NPAD), np.int64)
    n_lo_s[ncore, sortpos] = n_lo
    n_hi_s[ncore, sortpos] = n_hi
    K1 = n_lo_s.reshape(NCORES, NTILE, 128).max(axis=(0, 2))   # [NTILE]
    K2 = n_hi_s.reshape(NCORES, NTILE, 128).max(axis=(0, 2))
    KT = K1 + K2

    # rank of each edge within its (dst, half) group
    eorder = np.lexsort((~is_lo, d_sp, d_core))
    so_key = key[eorder]
    firsts = np.r_[True, so_key[1:] != so_key[:-1]]
    grp_start = np.maximum.accumulate(np.where(firsts, np.arange(len(eorder)), 0))
    rank_sorted = np.arange(len(eorder)) - grp_start
    rank = np.empty(len(eorder), np.int64)
    rank[eorder] = rank_sorted

    # flat slot column for each edge: tile column base + (rank or K1+rank)
    coff = np.zeros(NTILE + 1, np.int64)
    coff[1:] = np.cumsum(KT)
    tot_slots = int(coff[-1])
    tile_of = d_sp // 128
    p_of = d_sp % 128
    k_of = rank + np.where(is_lo, 0, K1[tile_of])

    # per-core idx arrays (value in table space); padding slots point at the
    # sentinel rows (alpha_src = MASK_NEG, h = 0) so no mask is needed
    idx_val = np.empty((NCORES, tot_slots, 128), np.int16)   # [core, col, p]
    for t in range(NTILE):
        idx_val[:, coff[t]:coff[t] + K1[t], :] = SENT_LO
        idx_val[:, coff[t] + K1[t]:coff[t + 1], :] = SENT_HI
    cols = coff[tile_of] + k_of
    val = np.where(is_lo, s_sid, s_sid - HIBASE).astype(np.int16)
    idx_val[d_core, cols, p_of] = val

    # gather instruction schedule (static, shared by all cores):
    # one lo + one hi gather per tile (batched; single_packet=False)
    sched = []          # (tile, half, slot_off_in_tile, nslots, idx_flat_off)
    off = 0
    for t in range(NTILE):
        for half, kk in ((0, int(K1[t])), (1, int(K2[t]))):
            if kk == 0:
                continue
            s0 = 0 if half == 0 else int(K1[t])
            sched.append((t, half, s0, kk, off))
            off += kk * 128
    tot_idx = off

    # flat idx buffers in j = k*128 + p order per chunk
    idx_wrapped = np.empty((NCORES, 128, tot_idx // 16), np.int16)
    for c in range(NCORES):
        flat = np.empty(tot_idx, np.int16)
        for (t, half, s0, nk, o) in sched:
            colbase = coff[t] + s0
            flat[o:o + nk * 128] = idx_val[c, colbase:colbase + nk].reshape(-1)
        idx_wrapped[c] = _wrap16(flat)

    # pooling: per-graph slot segments (shared static layout across cores:
    # graph g gets max-over-cores count slots; defaults hit the zero row).
    # batch is sorted, so a graph lives on 1-2 cores and gcnt ~= its largest
    # per-core piece; graphs are packed into blocks to bound SBUF.
    cnt_gc = np.zeros((G, NCORES), np.int64)
    for c in range(NCORES):
        cnt_gc[:, c] = np.bincount(batch[ncore == c], minlength=G)
    gcnt = cnt_gc.max(axis=1).astype(np.int64)               # [G]
    BLK_SLOTS = 6144
    gblocks = []          # list of (first_graph, ngraphs, nslots_padded)
    g0 = 0
    while g0 < G:
        g1, tot = g0, 0
        while g1 < G and (g1 == g0 or tot + gcnt[g1] <= BLK_SLOTS):
            tot += int(gcnt[g1])
            g1 += 1
        gblocks.append((g0, g1 - g0, (tot + 127) // 128 * 128))
        g0 = g1
    goff = np.zeros(G + 1, np.int64)      # slot offset of graph g in layout
    pos = 0
    gi = 0
    for (gb0, ng, npad_) in gblocks:
        run = pos
        for g_ in range(gb0, gb0 + ng):
            goff[g_] = run
            run += int(gcnt[g_])
        pos += npad_
    npool = int(pos)
    # slot value = sorted-local node row in the h3 staging, NPAD = zero row
    pool_idx = np.full((NCORES, npool), NPAD, np.int16)
    for c in range(NCORES):
        nodes_c = np.flatnonzero(ncore == c)
        gl = batch[nodes_c]
        order2 = np.argsort(gl, kind="stable")
        ranks = np.arange(NLOC) - np.maximum.accumulate(
            np.where(np.r_[True, gl[order2][1:] != gl[order2][:-1]],
                     np.arange(NLOC), 0))
        slots = goff[gl[order2]] + ranks
        pool_idx[c, slots] = sortpos[nodes_c[order2]]
    pool_wrapped = np.stack([_wrap16(pool_idx[c]) for c in range(NCORES)])

    # ---- weights / constants ----
    def aflat(a):
        m = np.zeros((128, H), np.float32)
        for h_ in range(H):
            m[h_ * C:(h_ + 1) * C, h_] = np.asarray(a, np.float32)[h_]
        return m

    W0 = np.asarray(inputs["W0"], np.float32)
    h0 = x @ W0
    h0r = h0.reshape(N, H, C)
    as0 = (h0r * np.asarray(inputs["as0"], np.float32)).sum(-1)
    ad0 = (h0r * np.asarray(inputs["ad0"], np.float32)).sum(-1)
    table0 = _pack_table_rows(h0, as0, sid, TROWS)

    # layer-0 alpha_dst per core in sorted order [128, NTILE*H]
    tmp = np.zeros((NCORES, NPAD, H), np.float32)
    tmp[ncore, sortpos] = ad0
    ad0_loc = np.ascontiguousarray(
        tmp.reshape(NCORES, NTILE, 128, H).transpose(0, 2, 1, 3).reshape(
            NCORES, 128, NTILE * H))

    consts = {
        "idx": idx_wrapped,                                  # per-core
        "pool_idx": pool_wrapped,                            # per-core
        "table0": table0,                                    # replicated
        "ad0_loc": ad0_loc,                                  # per-core
        "W1": np.asarray(inputs["W1"], np.float32),
        "W2": np.asarray(inputs["W2"], np.float32),
        # alphas are dot products against h = x@W, so fold W in:
        "afs1": np.asarray(inputs["W1"], np.float32) @ aflat(inputs["as1"]),
        "afd1": np.asarray(inputs["W1"], np.float32) @ aflat(inputs["ad1"]),
        "afs2": np.asarray(inputs["W2"], np.float32) @ aflat(inputs["as2"]),
        "afd2": np.asarray(inputs["W2"], np.float32) @ aflat(inputs["ad2"]),
        # additive alpha_src sentinel for the last tile's pad partitions
        "padsent": np.where(np.arange(128)[:, None] >= PADP0,
                            np.float32(MASK_NEG), np.float32(0.0))
                     * np.ones((128, H), np.float32),
        "b0c": np.tile(np.asarray(inputs["b0"], np.float32), (128, 1)),
        "b1c": np.tile(np.asarray(inputs["b1"], np.float32), (128, 1)),
        "b2c": np.tile(np.asarray(inputs["b2"], np.float32), (128, 1)),
        "identf": np.eye(128, dtype=np.float32),
        "identb": np.eye(128, dtype=np.float32),   # cast to bf16 at feed time
        "Wlin": np.asarray(inputs["Wlin"], np.float32),
        "blinc": np.tile(np.asarray(inputs["blin"], np.float32), (64, 1)),
        "Wout": np.asarray(inputs["Wout"], np.float32),
        "boutc": np.tile(np.asarray(inputs["bout"], np.float32), (64, 1)),
    }
    meta = dict(K1=[int(v) for v in K1], K2=[int(v) for v in K2],
                KT=[int(v) for v in KT], coff=[int(v) for v in coff],
                sched=sched, tot_idx=tot_idx, tot_slots=tot_slots,
                gcnt=[int(v) for v in gcnt], goff=[int(v) for v in goff],
                gblocks=gblocks, npool=npool)
    return consts, meta


# ---------------------------------------------------------------------------
def _build_module(meta):
    import concourse.bacc as bacc
    import concourse.bass as bass
    import concourse.mybir as mybir
    import concourse.tile as tile

    dtf = mybir.dt.float32
    dtb = mybir.dt.float16
    AF = mybir.ActivationFunctionType
    K1, K2, KT = meta["K1"], meta["K2"], meta["KT"]
    coff, sched = meta["coff"], meta["sched"]
    KMAX = max(KT)
    gcnt, goff, npool = meta["gcnt"], meta["goff"], meta["npool"]
    gblocks = meta["gblocks"]
    CWORDS = 66          # compact slab row: 64 words h fp16 + 2 words a_s fp16

    nc = bacc.Bacc("TRN2", target_bir_lowering=False, debug=False,
                   num_devices=NCORES)

    # ---- I/O ----
    t_idx = nc.dram_tensor("idx", [128, meta["tot_idx"] // 16], mybir.dt.int16,
                           kind="ExternalInput")
    t_pool = nc.dram_tensor("pool_idx", [128, npool // 16], mybir.dt.int16,
                            kind="ExternalInput")
    t_tab0 = nc.dram_tensor("table0", [TROWS, 128], dtf, kind="ExternalInput")
    t_ad0 = nc.dram_tensor("ad0_loc", [128, NTILE * H], dtf, kind="ExternalInput")
    t_identb = nc.dram_tensor("identb", [128, 128], dtb, kind="ExternalInput")
    t_identf = nc.dram_tensor("identf", [128, 128], dtf, kind="ExternalInput")
    ins = {}
    for nm, shp in (("W1", [128, 128]), ("W2", [128, 128]),
                    ("afs1", [128, H]), ("afd1", [128, H]),
                    ("afs2", [128, H]), ("afd2", [128, H]),
                    ("padsent", [128, H]),
                    ("b0c", [128, 128]), ("b1c", [128, 128]), ("b2c", [128, 128]),
                    ("Wlin", [128, LIN]), ("blinc", [64, LIN]),
                    ("Wout", [LIN, OUTD]), ("boutc", [64, OUTD])):
        ins[nm] = nc.dram_tensor(nm, shp, dtf, kind="ExternalInput")
    t_out = nc.dram_tensor("out", [64, OUTD], dtf, kind="ExternalOutput")
    dbg = {}
    if DEBUG_OUTS:
        for l_ in range(3):
            dbg[f"x1_l{l_}"] = nc.dram_tensor(f"dbg_x1_l{l_}", [NPAD, 128], dtf,
                                              kind="ExternalOutput")
        dbg["tab1"] = nc.dram_tensor("dbg_tab1", [NPAD, 128], dtf,
                                     kind="ExternalOutput")
        dbg["ad1"] = nc.dram_tensor("dbg_ad1", [128, NTILE * H], dtf,
                                    kind="ExternalOutput")
        dbg["pooledT"] = nc.dram_tensor("dbg_pooledT", [128, 64], dtf,
                                        kind="ExternalOutput")

    with tile.TileContext(nc) as tc:
        with (
            tc.tile_pool(name="const", bufs=1) as constp,
            tc.tile_pool(name="gslab", bufs=4) as gpool,
            tc.tile_pool(name="vslab", bufs=3) as vpool,
            tc.tile_pool(name="small", bufs=3) as spool,
            tc.tile_pool(name="node", bufs=3) as npool_sb,
            tc.tile_pool(name="keep", bufs=1) as keep,
            tc.tile_pool(name="gpp", bufs=2) as gpp,
            tc.tile_pool(name="ps", bufs=2, space="PSUM") as psp,
            tc.tile_pool(name="ps2", bufs=1, space="PSUM") as psp2,
            tc.tile_pool(name="dram", bufs=1, space="DRAM") as dram,
        ):
            # ---- persistent SBUF ----
            idx_sb = constp.tile([128, meta["tot_idx"] // 16], mybir.dt.int16)
            nc.sync.dma_start(idx_sb[:], t_idx[:])
            pool_sb = constp.tile([128, npool // 16], mybir.dt.int16)
            nc.sync.dma_start(pool_sb[:], t_pool[:])
            ident_b = constp.tile([128, 128], dtb)
            ident_f = constp.tile([128, 128], dtf)
            nc.sync.dma_start(ident_b[:], t_identb[:])
            nc.sync.dma_start(ident_f[:], t_identf[:])
            csb = {}
            for nm in ("W1", "W2", "afs1", "afd1", "afs2", "afd2", "padsent",
                       "b0c", "b1c", "b2c", "Wlin", "blinc", "boutc"):
                csb[nm] = constp.tile(list(ins[nm].shape), dtf, tag=nm, name=nm)
                nc.sync.dma_start(csb[nm][:], ins[nm][:])
            wout_sb = []
            for j in range(2):
                w = constp.tile([128, OUTD], dtf, tag=f"wout{j}", name=f"wout{j}")
                nc.sync.dma_start(w[:], ins["Wout"][j * 128:(j + 1) * 128, :])
                wout_sb.append(w)

            # alpha_dst for current layer, [128, NTILE*H]
            ad_cur = keep.tile([128, NTILE * H], dtf, tag="ad_cur")
            nc.sync.dma_start(ad_cur[:], t_ad0[:])
            ad_next = keep.tile([128, NTILE * H], dtf, tag="ad_next")

            # ---- DRAM tables ----
            tables = [t_tab0]
            slabs = []
            tabcs = []
            for l_ in (1, 2):
                tables.append(dram.tile([TROWS, 128], dtf,
                                        tag=f"tab{l_}", name=f"tab{l_}"))
                tabcs.append(dram.tile([TROWS, CWORDS], dtf,
                                       addr_space="Shared",
                                       tag=f"tabc{l_}", name=f"tabc{l_}"))
                slabs.append(dram.tile([NPAD, CWORDS], dtf, tag=f"slab{l_}",
                                       name=f"slab{l_}"))
            h3_stage = dram.tile([NPAD + 128, 128], dtb, tag="h3st",
                                 name="h3st")

            layer_w = {0: ("W1", "afs1", "afd1", "b0c"),
                       1: ("W2", "afs2", "afd2", "b1c"),
                       2: (None, None, None, "b2c")}

            for l_ in range(3):
                tabl = tables[l_]
                wname, asname, adname, bname = layer_w[l_]
                sched_by_tile = {}
                for (t, half, s0, nk, o) in sched:
                    sched_by_tile.setdefault(t, []).append((half, s0, nk, o))

                for t in range(NTILE):
                    kt, k1 = KT[t], K1[t]
                    # ---- gather payload rows into ELL slab ----
                    Gt = gpool.tile([128, KMAX, 128], dtf, tag="G")
                    for (half, s0, nk, o) in sched_by_tile[t]:
                        srcap = tabl[:LOCUT, :] if half == 0 else tabl[HIBASE:, :]
                        nc.gpsimd.dma_gather(
                            out_ap=Gt[:, s0:s0 + nk, :],
                            in_ap=srcap,
                            idxs_ap=idx_sb[:, o // 16:(o + nk * 128) // 16],
                            num_idxs=nk * 128,
                            num_idxs_reg=nk * 128,
                            elem_size=128,
                            single_packet=(nk * 128 <= 1024),
                        )
                    # views
                    g_bf = Gt[:].bitcast(dtb)                  # [128, KMAX, 256]
                    h_view = g_bf[:, 0:kt, 0:128].rearrange(
                        "p k (h g u) -> p k h g u", h=H, u=2)
                    as_view = g_bf[:, 0:kt, 128:128 + H]       # fp16 [128,kt,4]

                    # ---- attention (padding slots hit the sentinel rows,
                    # whose alpha_src = -60000 -> exp == 0, no mask) ----
                    e_t = spool.tile([128, KMAX, H], dtf, tag="e")
                    ad_b = ad_cur[:, t * H:(t + 1) * H].rearrange(
                        "p (u h) -> p u h", u=1).broadcast_to([128, kt, H])
                    nc.vector.tensor_tensor(
                        out=e_t[:, 0:kt, :], in0=as_view, in1=ad_b,
                        op=mybir.AluOpType.add)
                    # LeakyReLU = max(x, 0.2x) on DVE (ACT Lrelu table bakes
                    # its own slope and ignores alpha)
                    lr_t = spool.tile([128, KMAX, H], dtf, tag="lr")
                    nc.vector.tensor_scalar_mul(lr_t[:, 0:kt, :],
                                                e_t[:, 0:kt, :], NEG)
                    nc.vector.tensor_tensor(out=e_t[:, 0:kt, :],
                                            in0=e_t[:, 0:kt, :],
                                            in1=lr_t[:, 0:kt, :],
                                            op=mybir.AluOpType.max)
                    mx = spool.tile([128, H], dtf, tag="mx")
                    nc.vector.tensor_reduce(
                        out=mx[:],
                        in_=e_t[:, 0:kt, :].rearrange("p k h -> p h k"),
                        axis=mybir.AxisListType.X, op=mybir.AluOpType.max)
                    mx_b = mx[:].rearrange("p (u h) -> p u h", u=1).broadcast_to(
                        [128, kt, H])
                    nc.vector.tensor_tensor(out=e_t[:, 0:kt, :],
                                            in0=e_t[:, 0:kt, :], in1=mx_b,
                                            op=mybir.AluOpType.subtract)
                    # exp, written as interleaved pairs so the weighting
                    # multiply below qualifies for the 2x DVE mode (its last
                    # free dim is packed, the 16-wide broadcast sits mid-AP)
                    ex2 = spool.tile([128, KMAX, H, 2], dtb, tag="ex2")
                    e_b2 = e_t[:, 0:kt, :].rearrange(
                        "p k (h u) -> p k h u", u=1).broadcast_to(
                        [128, kt, H, 2])
                    nc.scalar.activation(ex2[:, 0:kt, :, :], e_b2, AF.Exp)

                    # ---- denominators ----
                    den = spool.tile([128, H], dtf, tag="den")
                    nc.vector.tensor_reduce(
                        out=den[:],
                        in_=ex2[:, 0:kt, :, 0].rearrange("p k h -> p h k"),
                        axis=mybir.AxisListType.X, op=mybir.AluOpType.add)
                    nc.vector.tensor_scalar_max(den[:], den[:], 1e-30)
                    rec = spool.tile([128, H], dtf, tag="rec")
                    nc.vector.reciprocal(rec[:], den[:])

                    # ---- weighted values (2x DVE mode) ----
                    v_t = vpool.tile([128, KMAX, H, C], dtb, tag="v")
                    ex_b = ex2[:, 0:kt, :, :].rearrange(
                        "p k h (o u) -> p k h o u", o=1).broadcast_to(
                        [128, kt, H, C // 2, 2])
                    v_view = v_t[:, 0:kt, :, :].rearrange(
                        "p k h (g u) -> p k h g u", u=2)
                    nc.vector.tensor_tensor(out=v_view,
                                            in0=h_view, in1=ex_b,
                                            op=mybir.AluOpType.mult)

                    # ---- aggregation: Num[d, hc] = sum_k v_k ----
                    num_ps = psp.tile([128, 128], dtf, tag="num")
                    vflat = v_t[:].rearrange("p k h c -> p k (h c)")
                    for k in range(kt):
                        nc.tensor.matmul(num_ps[:], lhsT=ident_b[:],
                                         rhs=vflat[:, k, :],
                                         start=(k == 0), stop=(k == kt - 1))

                    # ---- normalize + bias + relu -> x1 [d, hc] f32 ----
                    x1 = npool_sb.tile([128, 128], dtf, tag="x1")
                    rec_b = rec[:].rearrange("p (h u) -> p h u", u=1).broadcast_to(
                        [128, H, C])
                    nc.vector.tensor_tensor(
                        out=x1[:].rearrange("p (h c) -> p h c", h=H),
                        in0=num_ps[:].rearrange("p (h c) -> p h c", h=H),
                        in1=rec_b, op=mybir.AluOpType.mult)
                    nc.vector.tensor_tensor(out=x1[:], in0=x1[:],
                                            in1=csb[bname][:],
                                            op=mybir.AluOpType.add)
                    nc.vector.tensor_scalar_max(x1[:], x1[:], 0.0)
                    if DEBUG_OUTS:
                        nc.sync.dma_start(
                            dbg[f"x1_l{l_}"][t * 128:(t + 1) * 128, :], x1[:])

                    if l_ < 2:
                        # ---- transpose x1 -> x1T [hc, d] ----
                        x1T_ps = psp2.tile([128, 128], dtf, tag="x1T")
                        nc.tensor.matmul(x1T_ps[:], lhsT=x1[:],
                                         rhs=ident_f[:], start=True, stop=True)
                        x1T = npool_sb.tile([128, 128], dtf, tag="x1T_sb")
                        nc.scalar.copy(x1T[:], x1T_ps[:])
                        # ---- node pass: table row for next layer ----
                        row_ps = psp.tile([128, 136], dtf, tag="row")
                        nc.tensor.matmul(row_ps[:, 0:128], lhsT=x1T[:],
                                         rhs=csb[wname][:], start=True, stop=True)
                        nc.tensor.matmul(row_ps[:, 128:132], lhsT=x1T[:],
                                         rhs=csb[asname][:], start=True, stop=True)
                        nc.tensor.matmul(row_ps[:, 132:136], lhsT=x1T[:],
                                         rhs=csb[adname][:], start=True, stop=True)
                        slab_sb = npool_sb.tile([128, CWORDS], dtf, tag="slabrow")
                        slab_bf = slab_sb[:].bitcast(dtb)      # [128, 132]
                        nc.scalar.copy(slab_bf[:, 0:132], row_ps[:, 0:132])
                        if t == NTILE - 1:
                            # pad rows double as sentinels: alpha_src=-60000
                            nc.vector.tensor_tensor(
                                out=slab_bf[:, 128:132],
                                in0=slab_bf[:, 128:132],
                                in1=csb["padsent"][:],
                                op=mybir.AluOpType.add)
                        nc.vector.tensor_copy(
                            ad_next[:, t * H:(t + 1) * H], row_ps[:, 132:136])
                        nc.sync.dma_start(
                            slabs[l_][t * 128:(t + 1) * 128, :], slab_sb[:])
                    else:
                        x1h = npool_sb.tile([128, 128], dtb, tag="x1h")
                        nc.scalar.copy(x1h[:], x1[:])
                        nc.sync.dma_start(
                            h3_stage[t * 128:(t + 1) * 128, :], x1h[:])

                if l_ < 2:
                    nc.gpsimd.collective_compute(
                        "AllGather", mybir.AluOpType.bypass,
                        replica_groups=[list(range(NCORES))],
                        ins=[slabs[l_][:]],
                        outs=[tabcs[l_][:]],
                    )
                    # expand compact 264B rows into the 512B-stride table
                    nc.sync.dma_start(tables[l_ + 1][:, 0:CWORDS],
                                      tabcs[l_][:])
                    if DEBUG_OUTS and l_ == 0:
                        nc.gpsimd.dma_start(dbg["tab1"][:],
                                            tables[1][0:NPAD, :])
                        dsb = keep.tile([128, NTILE * H], dtf, tag="dbgad")
                        nc.vector.tensor_copy(dsb[:], ad_next[:])
                        nc.sync.dma_start(dbg["ad1"][:], dsb[:])
                    ad_cur, ad_next = ad_next, ad_cur

            # ---------------- pooling + MLP ----------------
            # transpose-gather node columns [hc, slot] grouped by graph, then
            # per-graph max over each static slot segment.  Defaults hit the
            # zero row; x1 is post-relu >= 0, so zero slots are harmless.
            zrow = keep.tile([128, 128], dtb, tag="zrow")
            nc.vector.memset(zrow[:], 0.0)
            nc.sync.dma_start(h3_stage[NPAD:NPAD + 128, :], zrow[:])
            pooledT = keep.tile([128, 64], dtf, tag="pooledT")
            BMAX = max(b[2] for b in gblocks)
            pos = 0
            for (gb0, ng, nslot) in gblocks:
                gt = gpp.tile([128, BMAX], dtb, tag="gpool")
                gt3 = gt[:, 0:nslot].rearrange("p (o n) -> p o n", o=1)
                nc.gpsimd.dma_gather(
                    out_ap=gt3,
                    in_ap=h3_stage[:],
                    idxs_ap=pool_sb[:, pos // 16:(pos + nslot) // 16],
                    num_idxs=nslot, num_idxs_reg=nslot,
                    elem_size=128, transpose=True, single_packet=False)
                for g_ in range(gb0, gb0 + ng):
                    if gcnt[g_] == 0:
                        nc.vector.memset(pooledT[:, g_:g_ + 1], 0.0)
                    else:
                        a0 = goff[g_] - pos
                        nc.vector.tensor_reduce(
                            out=pooledT[:, g_:g_ + 1],
                            in_=gt[:, a0:a0 + gcnt[g_]],
                            axis=mybir.AxisListType.X, op=mybir.AluOpType.max)
                pos += nslot

            if DEBUG_OUTS:
                nc.sync.dma_start(dbg["pooledT"][:], pooledT[:])
            # cross-core max
            ar_in = dram.tile([128, 64], dtf, tag="arin")
            ar_out = dram.tile([128, 64], dtf, addr_space="Shared", tag="arout")
            nc.sync.dma_start(ar_in[:], pooledT[:])
            nc.gpsimd.collective_compute(
                "AllReduce", mybir.AluOpType.max,
                replica_groups=[list(range(NCORES))],
                ins=[ar_in.opt()], outs=[ar_out.opt()],
            )
            pooled_sb = keep.tile([128, 64], dtf, tag="pooled2")
            nc.sync.dma_start(pooled_sb[:], ar_out[:])

            # z = pooled @ Wlin + blin  -> [64, 256]
            z_ps = psp2.tile([64, LIN], dtf, tag="z")
            nc.tensor.matmul(z_ps[:], lhsT=pooled_sb[:],
                             rhs=csb["Wlin"][:], start=True, stop=True)
            z_sb = keep.tile([64, LIN], dtf, tag="zsb")
            nc.vector.tensor_tensor(out=z_sb[:], in0=z_ps[:],
                                    in1=csb["blinc"][:],
                                    op=mybir.AluOpType.add)
            # zT (two 128-chunks)
            out_ps = psp2.tile([64, OUTD], dtf, tag="o")
            for j in range(2):
                zT_ps = psp2.tile([128, 64], dtf, tag="zT")
                nc.tensor.matmul(zT_ps[:], lhsT=z_sb[:, j * 128:(j + 1) * 128],
                                 rhs=ident_f[0:64, 0:64], start=True, stop=True)
                zT_sb = keep.tile([128, 64], dtf, tag=f"zTsb{j}")
                nc.scalar.copy(zT_sb[:], zT_ps[:])
                nc.tensor.matmul(out_ps[:], lhsT=zT_sb[:],
                                 rhs=wout_sb[j][:], start=(j == 0), stop=(j == 1))
            out_sb = keep.tile([64, OUTD], dtf, tag="osb")
            nc.vector.tensor_tensor(out=out_sb[:], in0=out_ps[:],
                                    in1=csb["boutc"][:], op=mybir.AluOpType.add)
            nc.sync.dma_start(t_out[:], out_sb[:])

    nc.compile()
    return nc


def kernel(**inputs):
    consts, meta = _host_prep(inputs)

    key = (meta["tot_idx"], meta["tot_slots"], meta["npool"],
           tuple(meta["gcnt"]), tuple(meta["KT"]))
    if key not in _COMPILED:
        _COMPILED[key] = _build_module(meta)
    nc = _COMPILED[key]

    in_maps = []
    for c in range(NCORES):
        m = {}
        for nm, v in consts.items():
            if nm in ("idx", "pool_idx", "ad0_loc"):
                m[nm] = np.ascontiguousarray(v[c])
            else:
                m[nm] = v
        m["identb"] = m["identb"].astype(np.float16)
        in_maps.append(m)

    from concourse import bass2jax
    res = bass2jax.run_bass_via_pjrt(nc, in_maps, n_cores=NCORES)
    return np.asarray(res[0]["out"], np.float32)

